# revision 30
# baseline (speedup 1.0000x reference)
"""Causal self-attention on 8 TRN2 NeuronCores.

Reference computation (B=4, T=2048, C=1024, H=16 heads, hd=64):
    qkv = x @ W_attn + b_attn ; split q,k,v ; per-head causal softmax attention
    y = att @ v ; out = y @ W_proj + b_proj

Sharding: core = 2*b + g  (b = batch 0..3, g = head-half 0..1, heads 8g..8g+7).
Each core computes its batch's Q/K/V for its 8 heads, flash-style causal
attention entirely in SBUF, and a partial out^T = Wp_slice^T @ y^T.  The two
cores of a batch produce partials that the host sums (pure data movement +
one add); host also re-transposes to [B,T,C].

Layouts are feature-major (x^T, Q^T, K^T, y^T, out^T) so no transposes are
needed on device.  S^T[k,q] = K^T.T @ Q^T puts softmax on the partition axis;
the denominator comes free from an appended ones-column on V (M=65 matmul).
All matmul operands are bf16 (psum accumulation fp32): enables FWL weight
loads, halves DMA/SBUF traffic, and keeps rel-err ~2e-3 << 2e-2 budget.

Schedule: emission interleaves three streams so the PE never idles — qkv
projection of t-tile j+1 and earlier out-projections are woven between the
attention steps of q-wave j; within a wave the PV matmul of step k-LAG is
emitted next to the S matmul of step k.  A PE warm-up burst at t=0 takes the
HAM clock gate to 8/8 while the first DMAs land.  Softmax normalization is
deferred via a global step clock so its DMA round-trips never block the
strict-FIFO vector queue; the last wave's out-projection is split into a
c0-2 phase (interleaved with the final attention steps) and a c3 tail.
"""

import numpy as np

B, T, C, H = 4, 2048, 1024, 16
HD = C // H          # 64
HPC = 8              # heads per core
NCORES = 8
TQ = 512             # q tile (free dim / psum bank)
NQT = T // TQ        # 4 q tiles (waves) per batch
NCC = C // 128       # 8 contraction chunks of 128
LAG = 3              # PV lags S by this many attention steps
NWARM = 20           # PE warm-up matmuls (contiguous, N=512)
NKEEP = 36           # PE warm-keeper matmuls bridging the tail norm chain

_cache = {}


def _build():
    if "nc" in _cache:
        return _cache["nc"]

    import concourse.bass as bass
    import concourse.tile as tile
    from concourse import bacc, mybir

    F32 = mybir.dt.float32
    BF16 = mybir.dt.bfloat16
    AF = mybir.ActivationFunctionType

    nc = bacc.Bacc("TRN2", target_bir_lowering=False, debug=False,
                   num_devices=NCORES)

    F32R = mybir.dt.float32r

    xt_d = nc.dram_tensor("xt", [C, T], BF16, kind="ExternalInput").ap()
    wqk_d = nc.dram_tensor("wqk", [C, 1024], BF16, kind="ExternalInput").ap()
    wv_d = nc.dram_tensor("wv", [C, 512], BF16, kind="ExternalInput").ap()
    wp_d = nc.dram_tensor("wp", [512, C], BF16, kind="ExternalInput").ap()
    bqk_d = nc.dram_tensor("bqk", [1024], F32, kind="ExternalInput").ap()
    tri_d = nc.dram_tensor("tri", [128, 256], BF16, kind="ExternalInput").ap()
    eye_d = nc.dram_tensor("eye", [128, 128], F32, kind="ExternalInput").ap()
    outp_d = nc.dram_tensor("outp", [C, T], F32, kind="ExternalOutput").ap()

    with tile.TileContext(nc) as tc:
        import contextlib
        stack = contextlib.ExitStack()
        with stack:
            singles = stack.enter_context(tc.tile_pool(name="singles", bufs=1))
            ps = stack.enter_context(tc.tile_pool(name="ps", space="PSUM",
                                                  bufs=1))
            qpool = stack.enter_context(tc.tile_pool(name="qpool", bufs=2))
            ypool = stack.enter_context(tc.tile_pool(name="ypool", bufs=8))
            xtp = stack.enter_context(tc.tile_pool(name="xtp", bufs=2))
            ppool = stack.enter_context(tc.tile_pool(name="ppool", bufs=4))
            bcp = stack.enter_context(tc.tile_pool(name="bcp", bufs=6))
            ostp = stack.enter_context(tc.tile_pool(name="ostp", bufs=6))
            otap = stack.enter_context(tc.tile_pool(name="otap", bufs=1))
            ystg = stack.enter_context(tc.tile_pool(name="ystg", bufs=8))
            drp = stack.enter_context(tc.tile_pool(name="drp", bufs=16,
                                                   space="DRAM"))

            tri_sb = singles.tile([128, 2, 128], BF16)
            bqk_sb = singles.tile([128, 8], F32)

            # K^T resident: [feat128, pair, t].  V: [t128, kchunk, head, 128]
            # padded to 128 weight columns (cols 65-127 stay zero) so PV
            # matmuls get the compiler's fast-weight-load path; col 64 is
            # the ones-column that accumulates the softmax denominator.
            k_sb = singles.tile([128, 4, T], BF16)
            v_sb = singles.tile([128, T // 128, HPC, 128], BF16)
            ones_sb = singles.tile([128, (T // 128) * HPC], BF16)
            wqk_sb = singles.tile([128, NCC, 1024], BF16)
            wv_sb = singles.tile([128, NCC, 512], BF16)
            wp_sb = singles.tile([128, 4, 1024], BF16)
            eye_sb = singles.tile([128, 128], F32R)
            warm = singles.tile([1, 4], F32)
            ww = singles.tile([128, TQ], BF16)
            wqk_r = wqk_d.rearrange("(c p) m -> p c m", p=128)
            wv_r = wv_d.rearrange("(c p) m -> p c m", p=128)
            wp_r = wp_d.rearrange("(c p) m -> p c m", p=128)

            # Input DMA triggers first, spread over four engine queues —
            # descriptor generation costs ~600ns per trigger, so a single
            # queue serializes the head.  x on sync+scalar (needed first),
            # wqk on vector+gpsimd, wv on scalar behind x.
            xt0 = [xtp.tile([128, TQ], BF16, tag=f"xt{c}", name=f"xt_0_{c}")
                   for c in range(NCC)]
            xr0 = xt_d.rearrange("(c p) t -> p c t", p=128)[:, :, 0:TQ]
            # Priority order per queue: x and wv first (they gate the V
            # projection matmuls), wqk second — the HBM wire is the head
            # bottleneck, so later-needed bytes must not steal it.
            nc.vector.memset(ww, 0.0)
            nc.gpsimd.dma_start(
                out=tri_sb,
                in_=tri_d.rearrange("p (a q) -> p a q", a=2))
            nc.gpsimd.dma_start(
                out=bqk_sb, in_=bqk_d.rearrange("(c p) -> p c", p=128))
            nc.gpsimd.dma_start(out=eye_sb, in_=eye_d.bitcast(F32R))
            for c in range(3):
                nc.sync.dma_start(out=xt0[c], in_=xr0[:, c, :])
                nc.scalar.dma_start(out=xt0[3 + c], in_=xr0[:, 3 + c, :])
            nc.gpsimd.dma_start(out=xt0[6], in_=xr0[:, 6, :])
            nc.gpsimd.dma_start(out=xt0[7], in_=xr0[:, 7, :])
            for c in range(3):
                nc.sync.dma_start(out=wv_sb[:, c, :], in_=wv_r[:, c, :])
                nc.scalar.dma_start(out=wv_sb[:, 3 + c, :],
                                    in_=wv_r[:, 3 + c, :])
            nc.gpsimd.dma_start(out=wv_sb[:, 6, :], in_=wv_r[:, 6, :])
            nc.gpsimd.dma_start(out=wv_sb[:, 7, :], in_=wv_r[:, 7, :])
            for c in range(3):
                nc.sync.dma_start(out=wqk_sb[:, c, :], in_=wqk_r[:, c, :])
                nc.scalar.dma_start(out=wqk_sb[:, 3 + c, :],
                                    in_=wqk_r[:, 3 + c, :])
            nc.gpsimd.dma_start(out=wqk_sb[:, 6, :], in_=wqk_r[:, 6, :])
            nc.gpsimd.dma_start(out=wqk_sb[:, 7, :], in_=wqk_r[:, 7, :])

            # ACT table warm-up (exp set load) + PE HAM warm-up: one
            # gap-free accumulation chain keeps the PE array continuously
            # busy ~4us so the clock gate opens to 8/8 (the activity window
            # needs sustained busy) while the input DMAs land.
            nc.vector.memset(warm, 0.0)
            nc.scalar.activation(warm, warm, AF.Exp)
            wacc = ps.tile([128, TQ], F32, tag="acc", bufs=2, name="wm")
            for wi in range(NWARM):
                nc.tensor.matmul(wacc, ww[:, 0:128], ww, start=(wi == 0),
                                 stop=(wi == NWARM - 1))

            nc.vector.memset(v_sb[:, :, :, 65:128], 0.0)
            nc.vector.memset(ones_sb, 1.0)
            nc.vector.tensor_copy(
                v_sb[:, :, :, 64],
                ones_sb.rearrange("p (a b) -> p a b", a=T // 128))

            def load_wp():
                for c in range(4):
                    nc.gpsimd.dma_start(out=wp_sb[:, c, :], in_=wp_r[:, c, :])

            q_tiles = {}   # wave j -> [128, 4, TQ] tile
            y_tiles = {}   # keys (j, cp) psum pair, (j, "sb", cp) sbuf tile
            ota = {}       # wave-3 split projection partials

            # Global step clock + deferral queue: closures scheduled at a
            # future attention step so DMA-latency-bound ops never sit at
            # the head of the strict-FIFO vector/gpsimd queues.
            gstep = [0]
            deferred = []

            def defer(delta, fn):
                deferred.append([gstep[0] + delta, fn])

            def flush(now=None):
                i = 0
                while i < len(deferred):
                    due, fn = deferred[i]
                    if now is None or due <= now:
                        deferred.pop(i)
                        fn()
                        i = 0  # emitted fns may defer more
                    else:
                        i += 1

            # ---------- emission closures ----------
            def qkv_groups(tt):
                """13 emission closures for t-tile tt of the projections."""
                xt = [None]

                def load_x():
                    if tt == 0:
                        xt[0] = xt0
                    else:
                        xt[0] = [xtp.tile([128, TQ], BF16, tag=f"xt{c}",
                                          name=f"xt_{tt}_{c}")
                                 for c in range(NCC)]
                        xr = xt_d.rearrange("(c p) t -> p c t", p=128) \
                            [:, :, tt * TQ:(tt + 1) * TQ]
                        for c in range(NCC):
                            nc.sync.dma_start(out=xt[0][c], in_=xr[:, c, :])
                    q_tiles[tt] = qpool.tile([128, 4, TQ], BF16, tag="q",
                                             name=f"q_{tt}")

                def qk_chunk(m):
                    def emit():
                        acc = ps.tile([128, TQ], F32, tag="acc", bufs=2,
                                      name=f"acc_qk_{tt}_{m}")
                        for c in range(NCC):
                            nc.tensor.matmul(
                                acc, wqk_sb[:, c, m * 128:(m + 1) * 128],
                                xt[0][c],
                                start=(c == 0), stop=(c == NCC - 1))
                        if m < 4:
                            dst = q_tiles[tt][:, m, :]
                        else:
                            dst = k_sb[:, m - 4, tt * TQ:(tt + 1) * TQ]
                        nc.vector.tensor_scalar_add(dst, acc,
                                                    bqk_sb[:, m:m + 1])
                    return emit

                def v_chunk(v4):
                    def emit():
                        ti = tt * 4 + v4
                        acc = ps.tile([128, TQ], F32, tag="acc", bufs=2,
                                      name=f"acc_v_{tt}_{v4}")
                        for c in range(NCC):
                            nc.tensor.matmul(
                                acc, xt[0][c][:, v4 * 128:(v4 + 1) * 128],
                                wv_sb[:, c, :],
                                start=(c == 0), stop=(c == NCC - 1))
                        nc.vector.tensor_copy(
                            v_sb[:, ti, :, 0:64],
                            acc.rearrange("p (h d) -> p h d", h=HPC))
                    return emit

                # (early, late): early groups emit during wave tt-1 (they
                # gate wave tt's first steps); late groups spill into wave tt.
                early = [load_x]
                early += [v_chunk(v4) for v4 in range(4)]
                early += [qk_chunk(0), qk_chunk(4)]
                late = []
                for cpx in range(1, 4):
                    late += [qk_chunk(cpx), qk_chunk(4 + cpx)]
                return early, late

            def proj_groups(j):
                def chunk(mo):
                    def emit():
                        acc = ps.tile([128, TQ], F32, tag="acc", bufs=2,
                                      name=f"acc_pr_{j}_{mo}")
                        for c in range(4):
                            nc.tensor.matmul(
                                acc, wp_sb[:, c, mo * 128:(mo + 1) * 128],
                                y_tiles[(j, "sb", c)],
                                start=(c == 0), stop=(c == 3))
                        ot = ostp.tile([128, TQ], F32, tag="ot",
                                       name=f"ot_{j}_{mo}")
                        if mo % 2 == 1:
                            nc.scalar.copy(ot, acc)
                        else:
                            nc.vector.tensor_copy(ot, acc)
                        nc.sync.dma_start(
                            out=outp_d[mo * 128:(mo + 1) * 128,
                                       j * TQ:(j + 1) * TQ],
                            in_=ot)
                    return emit
                return [chunk(mo) for mo in range(8)]

            # Wave-3 split projection: phase A (contraction chunks 0-2)
            # interleaves with the last attention steps; phase B (chunk 3)
            # adds into the partials right after the final normalization.
            def proj3_A(mo):
                def emit():
                    acc = ps.tile([128, TQ], F32, tag="acc", bufs=2,
                                  name=f"acc_p3a_{mo}")
                    for c in range(3):
                        nc.tensor.matmul(
                            acc, wp_sb[:, c, mo * 128:(mo + 1) * 128],
                            y_tiles[(3, "sb", c)],
                            start=(c == 0), stop=(c == 2))
                    ota[mo] = otap.tile([128, TQ], F32R, tag=f"ota{mo}",
                                        name=f"ota_{mo}")
                    nc.vector.tensor_copy(ota[mo], acc)
                return emit

            def proj3_B():
                # psum += Wp_c3^T y3  then  psum += I @ otA  (PE-side add:
                # keeps the serialized DVE tensor_add off the tail).
                for mo in range(8):
                    acc = ps.tile([128, TQ], F32, tag="acc", bufs=2,
                                  name=f"acc_p3b_{mo}")
                    nc.tensor.matmul(
                        acc, wp_sb[:, 3, mo * 128:(mo + 1) * 128],
                        y_tiles[(3, "sb", 3)], start=True, stop=False)
                    nc.tensor.matmul(
                        acc, eye_sb, ota[mo],
                        start=False, stop=True)
                    ot = ostp.tile([128, TQ], F32, tag="ot",
                                   name=f"otb_{mo}")
                    if mo % 2 == 1:
                        nc.scalar.copy(ot, acc)
                    else:
                        nc.vector.tensor_copy(ot, acc)
                    nc.sync.dma_start(
                        out=outp_d[mo * 128:(mo + 1) * 128, 3 * TQ:4 * TQ],
                        in_=ot)

            def attention_wave(j):
                """Emission closures for q-wave j: pipelined S/exp/PV with
                LAG, plus deferred normalization per head-pair."""
                nkc = 4 * j + 4
                steps = [(cp, i) for cp in range(4) for i in range(nkc)]
                pend = {}
                mul_count = [0]

                def emit_S(k):
                    cp, i = steps[k]
                    r = max(0, (i - 4 * j) * 128)
                    s_ps = ps.tile([128, 2, TQ], F32, tag="s", bufs=2,
                                   name=f"s_{j}_{k}")
                    for par in range(2):
                        row0 = 64 * par
                        nc.tensor.matmul(
                            s_ps[:, par, r:TQ],
                            k_sb[row0:row0 + 64, cp, i * 128:(i + 1) * 128],
                            q_tiles[j][row0:row0 + 64, cp, r:TQ],
                            start=True, stop=True, tile_position=(row0, 0))
                    p_sb = ppool.tile([128, 2, TQ], BF16, tag="p",
                                      name=f"p_{j}_{k}")
                    nc.scalar.activation(p_sb[:, :, r:TQ], s_ps[:, :, r:TQ],
                                         AF.Exp)
                    if i >= 4 * j:
                        nc.vector.tensor_mul(p_sb[:, :, r:r + 128],
                                             p_sb[:, :, r:r + 128], tri_sb)
                    pend[k] = (r, p_sb)

                def emit_PV(k):
                    cp, i = steps[k]
                    r, p_sb = pend.pop(k)
                    if i == 0:
                        y_tiles[(j, cp)] = [
                            ps.tile([128, TQ], F32, tag="y", bufs=2,
                                    name=f"yps_{j}_{cp}_{par}")
                            for par in range(2)]
                    for par in range(2):
                        nc.tensor.matmul(
                            y_tiles[(j, cp)][par][:, r:TQ],
                            v_sb[:, i, 2 * cp + par, :], p_sb[:, par, r:TQ],
                            start=(i == 0), stop=(i == nkc - 1))
                    if i == nkc - 1:
                        norm_start(cp)

                def norm_start(cp):
                    # free the psum banks fast (one copy takes y + sums
                    # row), kick the transpose DMA, defer the rest.  The
                    # last chain of the kernel runs its DMA triggers on the
                    # (idle) sync queue — gpsimd triggers cost ~650ns each.
                    tail = (j == NQT - 1 and cp == 3)
                    if tail:
                        norm_tail()
                        return
                    for par in range(2):
                        y_ps = y_tiles[(j, cp)][par]
                        yst = ystg.tile([65, TQ], F32, tag="yst",
                                        name=f"yst_{j}_{cp}_{par}")
                        nc.vector.tensor_copy(yst, y_ps[0:65, :])
                        s4 = bcp.tile([128, 4], F32, tag="s4",
                                      name=f"s4_{j}_{cp}_{par}")
                        # parity chains split across queues: gpsimd alone
                        # saturates in the short waves
                        dmae = nc.gpsimd if par == 0 else nc.sync
                        dmae.dma_start(out=s4, in_=yst[64:65, :])
                        defer(2, mk_recip(cp, par, yst, s4))

                def norm_tail():
                    # Final head-pair: two-parity bounce chain with each
                    # parity's DMA hops on its own (idle) queue, plus
                    # warm-keeper matmuls so the PE clock gate stays open
                    # through the chain for the projection tail.
                    ysts = []
                    s4 = bcp.tile([128, 8], F32, tag="s4", name="s4_tail")
                    for par in range(2):
                        yst = ystg.tile([65, TQ], F32, tag="yst",
                                        name=f"yst_t_{par}")
                        nc.vector.tensor_copy(yst,
                                              y_tiles[(j, 3)][par][0:65, :])
                        ysts.append(yst)
                        eng = nc.sync if par == 0 else nc.scalar
                        eng.dma_start(out=s4[:, 4 * par:4 * par + 4],
                                      in_=yst[64:65, :])
                    keep = ps.tile([128, 2, TQ], F32, tag="s", bufs=2,
                                   name="keep_tail")
                    for wi in range(NKEEP):
                        nc.tensor.matmul(keep[:, 0, :], ww[:, 0:128], ww,
                                         start=(wi == 0),
                                         stop=(wi == NKEEP - 1))
                    r4 = bcp.tile([128, 8], F32, tag="r4", name="r4_tail")
                    nc.vector.reciprocal(r4, s4)
                    d2 = drp.tile([1, 2, TQ], F32, tag="d2", name="d2_tail")
                    bc = bcp.tile([64, 2, TQ], F32, tag="bc", name="bc_tail")
                    for par in range(2):
                        eng = nc.sync if par == 0 else nc.scalar
                        eng.dma_start(
                            out=bass.AP(tensor=d2.tensor,
                                        offset=d2.offset + par * TQ,
                                        ap=[[4, 128], [1, 4]]),
                            in_=r4[:, 4 * par:4 * par + 4])
                    for par in range(2):
                        eng = nc.sync if par == 0 else nc.scalar
                        eng.dma_start(
                            out=bc[:, par, :],
                            in_=bass.AP(tensor=d2.tensor,
                                        offset=d2.offset + par * TQ,
                                        ap=[[0, 64], [1, TQ]]))
                    for par in range(2):
                        row0 = 64 * par
                        nc.vector.tensor_mul(
                            y_tiles[(j, "sb", 3)][row0:row0 + 64, :],
                            ysts[par][0:64, :], bc[:, par, :])
                    proj3_B()

                def mk_recip(cp, par, yst, s4):
                    dmae = nc.gpsimd if par == 0 else nc.sync
                    def emit():
                        r4 = bcp.tile([128, 4], F32, tag="r4",
                                      name=f"r4_{j}_{cp}_{par}")
                        nc.vector.reciprocal(r4, s4)
                        d2 = drp.tile([1, TQ], F32, tag="d2",
                                      name=f"d2_{j}_{cp}_{par}")
                        dmae.dma_start(
                            out=bass.AP(tensor=d2.tensor, offset=d2.offset,
                                        ap=[[4, 128], [1, 4]]),
                            in_=r4)
                        bc = bcp.tile([64, TQ], F32, tag="bc",
                                      name=f"bc_{j}_{cp}_{par}")
                        dmae.dma_start(
                            out=bc,
                            in_=bass.AP(tensor=d2.tensor, offset=d2.offset,
                                        ap=[[0, 64], [1, TQ]]))
                        defer(2, mk_mul(cp, par, yst, bc))
                    return emit

                def mk_mul(cp, par, yst, bc):
                    def emit():
                        row0 = 64 * par
                        # deferred, so off the latency-critical paths; split
                        # across gpsimd/vector to keep both queues shallow
                        eng = nc.gpsimd if par == 0 else nc.vector
                        eng.tensor_mul(
                            y_tiles[(j, "sb", cp)][row0:row0 + 64, :],
                            yst[0:64, :], bc)
                        mul_count[0] += 1
                        if j == NQT - 1:
                            if mul_count[0] == 6:
                                for mo in range(8):
                                    defer(1 + mo, proj3_A(mo))
                            elif mul_count[0] == 8:
                                proj3_B()
                    return emit

                def step(k):
                    def emit():
                        gstep[0] += 1
                        flush(gstep[0])
                        if k == 0:
                            for cc in range(4):
                                y_tiles[(j, "sb", cc)] = ypool.tile(
                                    [128, TQ], BF16, tag="ysb",
                                    name=f"y_{j}_{cc}")
                        # PV first: the full-width PV pair follows the
                        # previous full-width matmul, and the row-tiled S
                        # pair absorbs any array-mode transition cost
                        if k >= LAG:
                            emit_PV(k - LAG)
                        if k < len(steps):
                            emit_S(k)
                    return emit

                return [step(k) for k in range(len(steps) + LAG)]

            # ---------- interleaved emission ----------
            g0_early, g0_late = qkv_groups(0)
            for fn in g0_early:
                fn()
            spill = list(g0_late)
            for j in range(NQT):
                attn = attention_wave(j)
                others = list(spill)
                spill = []
                if j == 0:
                    others.append(load_wp)
                if j + 1 < NQT:
                    early, late = qkv_groups(j + 1)
                    others += early
                    spill = late
                if j == 2:
                    others += proj_groups(0)
                if j == 3:
                    others += proj_groups(1) + proj_groups(2)
                done_o = 0
                frontier = max(1, (len(attn) * 3) // 5)
                for s, fn in enumerate(attn):
                    fn()
                    want = min(len(others), (s + 1) * len(others) // frontier)
                    while done_o < want:
                        others[done_o]()
                        done_o += 1
                while done_o < len(others):
                    others[done_o]()
                    done_o += 1
            flush()  # cascade any remaining deferred norm / projection work

    nc.compile()
    _cache["nc"] = nc
    return nc


def _prep_inputs(x, W_attn, b_attn, W_proj, b_proj):
    """Host-side sharding: returns in_maps for the 8 cores."""
    import ml_dtypes
    BF = ml_dtypes.bfloat16

    x = np.ascontiguousarray(np.asarray(x, dtype=np.float32))
    W_attn = np.asarray(W_attn, dtype=np.float32)
    b_attn = np.asarray(b_attn, dtype=np.float32)
    W_proj = np.asarray(W_proj, dtype=np.float32)
    b_proj = np.asarray(b_proj, dtype=np.float32)

    bv_full = b_attn[2 * C:3 * C]
    _cache["bout_host"] = (b_proj + bv_full @ W_proj).astype(np.float32)
    tri1 = np.triu(np.ones((128, 128), dtype=np.float32))  # 1 if k<=q
    tri = np.concatenate([tri1, tri1], axis=1).astype(BF)

    xts = [np.ascontiguousarray(x[b].T.astype(BF)) for b in range(B)]
    per_g = []
    for g in range(2):
        sl = slice(512 * g, 512 * (g + 1))
        wq = W_attn[:, 0:C][:, sl] * (1.0 / np.sqrt(HD))
        wk = W_attn[:, C:2 * C][:, sl]
        wv = W_attn[:, 2 * C:3 * C][:, sl]
        bq = b_attn[0:C][sl] * (1.0 / np.sqrt(HD))
        bk = b_attn[C:2 * C][sl]
        wp = W_proj[sl, :]
        per_g.append({
            "wqk": np.ascontiguousarray(
                np.concatenate([wq, wk], axis=1).astype(BF)),
            "wv": np.ascontiguousarray(wv.astype(BF)),
            "wp": np.ascontiguousarray(wp.astype(BF)),
            "bqk": np.ascontiguousarray(np.concatenate([bq, bk])),
        })

    eye = np.ascontiguousarray(np.eye(128, dtype=np.float32))
    in_maps = []
    for b in range(B):
        for g in range(2):
            m = dict(per_g[g])
            m["xt"] = xts[b]
            m["tri"] = tri
            m["eye"] = eye
            in_maps.append(m)
    return in_maps


def run_sharded(x, W_attn, b_attn, W_proj, b_proj, trace=False):
    """Run on 8 cores; returns (output [B,T,C], BassKernelResults)."""
    from concourse.bass_utils import run_bass_kernel_spmd

    nc = _build()
    in_maps = _prep_inputs(x, W_attn, b_attn, W_proj, b_proj)
    res = run_bass_kernel_spmd(nc, in_maps, list(range(NCORES)), trace=trace)
    outs = [res.results[i]["outp"] for i in range(NCORES)]
    bout = _cache["bout_host"]
    out = np.empty((B, T, C), dtype=np.float32)
    for b in range(B):
        out[b] = (outs[2 * b] + outs[2 * b + 1]).T + bout
    return out, res


def kernel(x, W_attn, b_attn, W_proj, b_proj):
    out, _ = run_sharded(x, W_attn, b_attn, W_proj, b_proj, trace=False)
    return out


# revision 31
# speedup vs baseline: 1.1150x; 1.1150x over previous
"""Causal self-attention on 8 TRN2 NeuronCores.

Reference computation (B=4, T=2048, C=1024, H=16 heads, hd=64):
    qkv = x @ W_attn + b_attn ; split q,k,v ; per-head causal softmax attention
    y = att @ v ; out = y @ W_proj + b_proj

Sharding: core = 2*b + g  (b = batch 0..3, g = head-half 0..1, heads 8g..8g+7).
Each core computes its batch's Q/K/V for its 8 heads, flash-style causal
attention entirely in SBUF, and a partial out^T = Wp_slice^T @ y^T.  The two
cores of a batch produce partials that the host sums (pure data movement +
one add); host also re-transposes to [B,T,C].

Layouts are feature-major (x^T, Q^T, K^T, y^T, out^T) so no transposes are
needed on device.  S^T[k,q] = K^T.T @ Q^T puts softmax on the partition axis;
the denominator comes free from an appended ones-column on V (M=65 matmul).
All matmul operands are bf16 (psum accumulation fp32): enables FWL weight
loads, halves DMA/SBUF traffic, and keeps rel-err ~2e-3 << 2e-2 budget.

Schedule: emission interleaves three streams so the PE never idles — qkv
projection of t-tile j+1 and earlier out-projections are woven between the
attention steps of q-wave j; within a wave the PV matmul of step k-LAG is
emitted next to the S matmul of step k.  A PE warm-up burst at t=0 takes the
HAM clock gate to 8/8 while the first DMAs land.  Softmax normalization is
deferred via a global step clock so its DMA round-trips never block the
strict-FIFO vector queue; the last wave's out-projection is split into a
c0-2 phase (interleaved with the final attention steps) and a c3 tail.
"""

import numpy as np

B, T, C, H = 4, 2048, 1024, 16
HD = C // H          # 64
HPC = 8              # heads per core
NCORES = 8
TQ = 512             # q tile (free dim / psum bank)
NQT = T // TQ        # 4 q tiles (waves) per batch
NCC = C // 128       # 8 contraction chunks of 128
LAG = 3              # PV lags S by this many attention steps
NWARM = 14           # PE warm-up matmuls (contiguous, N=512)
NKEEP = 36           # PE warm-keeper matmuls bridging the tail norm chain

_cache = {}


def _build():
    if "nc" in _cache:
        return _cache["nc"]

    import concourse.bass as bass
    import concourse.tile as tile
    from concourse import bacc, mybir

    F32 = mybir.dt.float32
    BF16 = mybir.dt.bfloat16
    AF = mybir.ActivationFunctionType

    nc = bacc.Bacc("TRN2", target_bir_lowering=False, debug=False,
                   num_devices=NCORES)

    F32R = mybir.dt.float32r

    xt_d = nc.dram_tensor("xt", [C, T], BF16, kind="ExternalInput").ap()
    wqk_d = nc.dram_tensor("wqk", [C, 1024], BF16, kind="ExternalInput").ap()
    wv_d = nc.dram_tensor("wv", [C, 512], BF16, kind="ExternalInput").ap()
    wp_d = nc.dram_tensor("wp", [512, C], BF16, kind="ExternalInput").ap()
    bqk_d = nc.dram_tensor("bqk", [1024], F32, kind="ExternalInput").ap()
    tri_d = nc.dram_tensor("tri", [128, 256], BF16, kind="ExternalInput").ap()
    eye_d = nc.dram_tensor("eye", [128, 128], F32, kind="ExternalInput").ap()
    outp_d = nc.dram_tensor("outp", [C, T], F32, kind="ExternalOutput").ap()

    with tile.TileContext(nc) as tc:
        import contextlib
        stack = contextlib.ExitStack()
        with stack:
            singles = stack.enter_context(tc.tile_pool(name="singles", bufs=1))
            ps = stack.enter_context(tc.tile_pool(name="ps", space="PSUM",
                                                  bufs=1))
            qpool = stack.enter_context(tc.tile_pool(name="qpool", bufs=2))
            ypool = stack.enter_context(tc.tile_pool(name="ypool", bufs=8))
            xtp = stack.enter_context(tc.tile_pool(name="xtp", bufs=2))
            ppool = stack.enter_context(tc.tile_pool(name="ppool", bufs=4))
            bcp = stack.enter_context(tc.tile_pool(name="bcp", bufs=6))
            ostp = stack.enter_context(tc.tile_pool(name="ostp", bufs=6))
            otap = stack.enter_context(tc.tile_pool(name="otap", bufs=1))
            ystg = stack.enter_context(tc.tile_pool(name="ystg", bufs=8))
            drp = stack.enter_context(tc.tile_pool(name="drp", bufs=16,
                                                   space="DRAM"))

            tri_sb = singles.tile([128, 2, 128], BF16)
            bqk_sb = singles.tile([128, 8], F32)

            # K^T resident: [feat128, pair, t].  V: [t128, kchunk, head, 128]
            # padded to 128 weight columns (cols 65-127 stay zero) so PV
            # matmuls get the compiler's fast-weight-load path; col 64 is
            # the ones-column that accumulates the softmax denominator.
            k_sb = singles.tile([128, 4, T], BF16)
            v_sb = singles.tile([128, T // 128, HPC, 128], BF16)
            ones_sb = singles.tile([128, (T // 128) * HPC], BF16)
            wqk_sb = singles.tile([128, NCC, 1024], BF16)
            wv_sb = singles.tile([128, NCC, 512], BF16)
            wp_sb = singles.tile([128, 4, 1024], BF16)
            eye_sb = singles.tile([128, 128], F32R)
            warm = singles.tile([1, 4], F32)
            ww = singles.tile([128, TQ], BF16)
            wqk_r = wqk_d.rearrange("(c p) m -> p c m", p=128)
            wv_r = wv_d.rearrange("(c p) m -> p c m", p=128)
            wp_r = wp_d.rearrange("(c p) m -> p c m", p=128)

            # Input DMA triggers first, spread over four engine queues —
            # descriptor generation costs ~600ns per trigger, so a single
            # queue serializes the head.  x on sync+scalar (needed first),
            # wqk on vector+gpsimd, wv on scalar behind x.
            xt0 = [xtp.tile([128, TQ], BF16, tag=f"xt{c}", name=f"xt_0_{c}")
                   for c in range(NCC)]
            xr0 = xt_d.rearrange("(c p) t -> p c t", p=128)[:, :, 0:TQ]
            # Priority order per queue: x and wv first (they gate the V
            # projection matmuls), wqk second — the HBM wire is the head
            # bottleneck, so later-needed bytes must not steal it.
            nc.vector.memset(ww, 0.0)
            nc.gpsimd.dma_start(
                out=tri_sb,
                in_=tri_d.rearrange("p (a q) -> p a q", a=2))
            nc.gpsimd.dma_start(
                out=bqk_sb, in_=bqk_d.rearrange("(c p) -> p c", p=128))
            nc.gpsimd.dma_start(out=eye_sb, in_=eye_d.bitcast(F32R))
            for c in range(3):
                nc.sync.dma_start(out=xt0[c], in_=xr0[:, c, :])
                nc.scalar.dma_start(out=xt0[3 + c], in_=xr0[:, 3 + c, :])
            nc.gpsimd.dma_start(out=xt0[6], in_=xr0[:, 6, :])
            nc.gpsimd.dma_start(out=xt0[7], in_=xr0[:, 7, :])
            for c in range(3):
                nc.sync.dma_start(out=wv_sb[:, c, :], in_=wv_r[:, c, :])
                nc.scalar.dma_start(out=wv_sb[:, 3 + c, :],
                                    in_=wv_r[:, 3 + c, :])
            nc.gpsimd.dma_start(out=wv_sb[:, 6, :], in_=wv_r[:, 6, :])
            nc.gpsimd.dma_start(out=wv_sb[:, 7, :], in_=wv_r[:, 7, :])
            for c in range(3):
                nc.sync.dma_start(out=wqk_sb[:, c, :], in_=wqk_r[:, c, :])
                nc.scalar.dma_start(out=wqk_sb[:, 3 + c, :],
                                    in_=wqk_r[:, 3 + c, :])
            nc.gpsimd.dma_start(out=wqk_sb[:, 6, :], in_=wqk_r[:, 6, :])
            nc.gpsimd.dma_start(out=wqk_sb[:, 7, :], in_=wqk_r[:, 7, :])

            # ACT table warm-up (exp set load) + PE HAM warm-up: one
            # gap-free accumulation chain keeps the PE array continuously
            # busy ~4us so the clock gate opens to 8/8 (the activity window
            # needs sustained busy) while the input DMAs land.
            nc.vector.memset(warm, 0.0)
            nc.scalar.activation(warm, warm, AF.Exp)
            wacc = ps.tile([128, TQ], F32, tag="acc", bufs=2, name="wm")
            for wi in range(NWARM):
                nc.tensor.matmul(wacc, ww[:, 0:128], ww, start=(wi == 0),
                                 stop=(wi == NWARM - 1))

            nc.vector.memset(v_sb[:, :, :, 65:128], 0.0)
            nc.vector.memset(ones_sb, 1.0)
            nc.vector.tensor_copy(
                v_sb[:, :, :, 64],
                ones_sb.rearrange("p (a b) -> p a b", a=T // 128))

            def load_wp():
                for c in range(4):
                    nc.gpsimd.dma_start(out=wp_sb[:, c, :], in_=wp_r[:, c, :])

            q_tiles = {}   # wave j -> [128, 4, TQ] tile
            y_tiles = {}   # keys (j, cp) psum pair, (j, "sb", cp) sbuf tile
            ota = {}       # wave-3 split projection partials

            # Global step clock + deferral queue: closures scheduled at a
            # future attention step so DMA-latency-bound ops never sit at
            # the head of the strict-FIFO vector/gpsimd queues.
            gstep = [0]
            deferred = []

            def defer(delta, fn):
                deferred.append([gstep[0] + delta, fn])

            def flush(now=None):
                i = 0
                while i < len(deferred):
                    due, fn = deferred[i]
                    if now is None or due <= now:
                        deferred.pop(i)
                        fn()
                        i = 0  # emitted fns may defer more
                    else:
                        i += 1

            # ---------- emission closures ----------
            def qkv_groups(tt):
                """13 emission closures for t-tile tt of the projections."""
                xt = [None]

                def load_x():
                    if tt == 0:
                        xt[0] = xt0
                    else:
                        xt[0] = [xtp.tile([128, TQ], BF16, tag=f"xt{c}",
                                          name=f"xt_{tt}_{c}")
                                 for c in range(NCC)]
                        xr = xt_d.rearrange("(c p) t -> p c t", p=128) \
                            [:, :, tt * TQ:(tt + 1) * TQ]
                        for c in range(NCC):
                            nc.sync.dma_start(out=xt[0][c], in_=xr[:, c, :])
                    q_tiles[tt] = qpool.tile([128, 4, TQ], BF16, tag="q",
                                             name=f"q_{tt}")

                def qk_chunk(m):
                    def emit():
                        acc = ps.tile([128, TQ], F32, tag="acc", bufs=2,
                                      name=f"acc_qk_{tt}_{m}")
                        for c in range(NCC):
                            nc.tensor.matmul(
                                acc, wqk_sb[:, c, m * 128:(m + 1) * 128],
                                xt[0][c],
                                start=(c == 0), stop=(c == NCC - 1))
                        if m < 4:
                            dst = q_tiles[tt][:, m, :]
                        else:
                            dst = k_sb[:, m - 4, tt * TQ:(tt + 1) * TQ]
                        nc.vector.tensor_scalar_add(dst, acc,
                                                    bqk_sb[:, m:m + 1])
                    return emit

                def v_chunk(v4):
                    def emit():
                        ti = tt * 4 + v4
                        acc = ps.tile([128, TQ], F32, tag="acc", bufs=2,
                                      name=f"acc_v_{tt}_{v4}")
                        for c in range(NCC):
                            nc.tensor.matmul(
                                acc, xt[0][c][:, v4 * 128:(v4 + 1) * 128],
                                wv_sb[:, c, :],
                                start=(c == 0), stop=(c == NCC - 1))
                        nc.vector.tensor_copy(
                            v_sb[:, ti, :, 0:64],
                            acc.rearrange("p (h d) -> p h d", h=HPC))
                    return emit

                # (early, late): early groups emit during wave tt-1 (they
                # gate wave tt's first steps); late groups spill into wave tt.
                early = [load_x]
                early += [v_chunk(v4) for v4 in range(4)]
                early += [qk_chunk(0), qk_chunk(4)]
                late = []
                for cpx in range(1, 4):
                    late += [qk_chunk(cpx), qk_chunk(4 + cpx)]
                return early, late

            def proj_groups(j):
                def chunk(mo):
                    def emit():
                        acc = ps.tile([128, TQ], F32, tag="acc", bufs=2,
                                      name=f"acc_pr_{j}_{mo}")
                        for c in range(4):
                            nc.tensor.matmul(
                                acc, wp_sb[:, c, mo * 128:(mo + 1) * 128],
                                y_tiles[(j, "sb", c)],
                                start=(c == 0), stop=(c == 3))
                        ot = ostp.tile([128, TQ], F32, tag="ot",
                                       name=f"ot_{j}_{mo}")
                        if mo % 2 == 1:
                            nc.scalar.copy(ot, acc)
                        else:
                            nc.vector.tensor_copy(ot, acc)
                        nc.sync.dma_start(
                            out=outp_d[mo * 128:(mo + 1) * 128,
                                       j * TQ:(j + 1) * TQ],
                            in_=ot)
                    return emit
                return [chunk(mo) for mo in range(8)]

            # Wave-3 split projection: phase A (contraction chunks 0-2)
            # interleaves with the last attention steps; phase B (chunk 3)
            # adds into the partials right after the final normalization.
            def proj3_A(mo):
                def emit():
                    acc = ps.tile([128, TQ], F32, tag="acc", bufs=2,
                                  name=f"acc_p3a_{mo}")
                    for c in range(3):
                        nc.tensor.matmul(
                            acc, wp_sb[:, c, mo * 128:(mo + 1) * 128],
                            y_tiles[(3, "sb", c)],
                            start=(c == 0), stop=(c == 2))
                    ota[mo] = otap.tile([128, TQ], F32R, tag=f"ota{mo}",
                                        name=f"ota_{mo}")
                    nc.vector.tensor_copy(ota[mo], acc)
                return emit

            def proj3_B():
                # psum += Wp_c3^T y3  then  psum += I @ otA  (PE-side add:
                # keeps the serialized DVE tensor_add off the tail).
                for mo in range(8):
                    acc = ps.tile([128, TQ], F32, tag="acc", bufs=2,
                                  name=f"acc_p3b_{mo}")
                    nc.tensor.matmul(
                        acc, wp_sb[:, 3, mo * 128:(mo + 1) * 128],
                        y_tiles[(3, "sb", 3)], start=True, stop=False)
                    nc.tensor.matmul(
                        acc, eye_sb, ota[mo],
                        start=False, stop=True)
                    ot = ostp.tile([128, TQ], F32, tag="ot",
                                   name=f"otb_{mo}")
                    if mo % 2 == 1:
                        nc.scalar.copy(ot, acc)
                    else:
                        nc.vector.tensor_copy(ot, acc)
                    nc.sync.dma_start(
                        out=outp_d[mo * 128:(mo + 1) * 128, 3 * TQ:4 * TQ],
                        in_=ot)

            def attention_wave(j):
                """Emission closures for q-wave j: pipelined S/exp/PV with
                LAG, plus deferred normalization per head-pair."""
                nkc = 4 * j + 4
                steps = [(cp, i) for cp in range(4) for i in range(nkc)]
                pend = {}
                mul_count = [0]

                def emit_S(k):
                    cp, i = steps[k]
                    r = max(0, (i - 4 * j) * 128)
                    s_ps = ps.tile([128, 2, TQ], F32, tag="s", bufs=2,
                                   name=f"s_{j}_{k}")
                    for par in range(2):
                        row0 = 64 * par
                        nc.tensor.matmul(
                            s_ps[:, par, r:TQ],
                            k_sb[row0:row0 + 64, cp, i * 128:(i + 1) * 128],
                            q_tiles[j][row0:row0 + 64, cp, r:TQ],
                            start=True, stop=True, tile_position=(row0, 0))
                    p_sb = ppool.tile([128, 2, TQ], BF16, tag="p",
                                      name=f"p_{j}_{k}")
                    nc.scalar.activation(p_sb[:, :, r:TQ], s_ps[:, :, r:TQ],
                                         AF.Exp)
                    if i >= 4 * j:
                        nc.vector.tensor_mul(p_sb[:, :, r:r + 128],
                                             p_sb[:, :, r:r + 128], tri_sb)
                    pend[k] = (r, p_sb)

                def emit_PV(k):
                    cp, i = steps[k]
                    r, p_sb = pend.pop(k)
                    if i == 0:
                        y_tiles[(j, cp)] = [
                            ps.tile([128, TQ], F32, tag="y", bufs=2,
                                    name=f"yps_{j}_{cp}_{par}")
                            for par in range(2)]
                    for par in range(2):
                        nc.tensor.matmul(
                            y_tiles[(j, cp)][par][:, r:TQ],
                            v_sb[:, i, 2 * cp + par, :], p_sb[:, par, r:TQ],
                            start=(i == 0), stop=(i == nkc - 1))
                    if i == nkc - 1:
                        norm_start(cp)

                def norm_start(cp):
                    # free the psum banks fast (one copy takes y + sums
                    # row), kick the transpose DMA, defer the rest.  The
                    # last chain of the kernel runs its DMA triggers on the
                    # (idle) sync queue — gpsimd triggers cost ~650ns each.
                    tail = (j == NQT - 1 and cp == 3)
                    if tail:
                        norm_tail()
                        return
                    for par in range(2):
                        y_ps = y_tiles[(j, cp)][par]
                        yst = ystg.tile([65, TQ], F32, tag="yst",
                                        name=f"yst_{j}_{cp}_{par}")
                        nc.vector.tensor_copy(yst, y_ps[0:65, :])
                        s4 = bcp.tile([128, 4], F32, tag="s4",
                                      name=f"s4_{j}_{cp}_{par}")
                        # parity chains split across queues: gpsimd alone
                        # saturates in the short waves
                        dmae = nc.gpsimd if par == 0 else nc.sync
                        dmae.dma_start(out=s4, in_=yst[64:65, :])
                        defer(2, mk_recip(cp, par, yst, s4))

                def norm_tail():
                    # Final head-pair: two-parity bounce chain with each
                    # parity's DMA hops on its own (idle) queue, plus
                    # warm-keeper matmuls so the PE clock gate stays open
                    # through the chain for the projection tail.
                    ysts = []
                    s4 = bcp.tile([128, 8], F32, tag="s4", name="s4_tail")
                    for par in range(2):
                        yst = ystg.tile([65, TQ], F32, tag="yst",
                                        name=f"yst_t_{par}")
                        nc.vector.tensor_copy(yst,
                                              y_tiles[(j, 3)][par][0:65, :])
                        ysts.append(yst)
                        eng = nc.sync if par == 0 else nc.scalar
                        eng.dma_start(out=s4[:, 4 * par:4 * par + 4],
                                      in_=yst[64:65, :])
                    keep = ps.tile([128, 2, TQ], F32, tag="s", bufs=2,
                                   name="keep_tail")
                    for wi in range(NKEEP):
                        nc.tensor.matmul(keep[:, 0, :], ww[:, 0:128], ww,
                                         start=(wi == 0),
                                         stop=(wi == NKEEP - 1))
                    r4 = bcp.tile([128, 8], F32, tag="r4", name="r4_tail")
                    nc.vector.reciprocal(r4, s4)
                    d2 = drp.tile([1, 2, TQ], F32, tag="d2", name="d2_tail")
                    bc = bcp.tile([64, 2, TQ], F32, tag="bc", name="bc_tail")
                    for par in range(2):
                        eng = nc.sync if par == 0 else nc.scalar
                        eng.dma_start(
                            out=bass.AP(tensor=d2.tensor,
                                        offset=d2.offset + par * TQ,
                                        ap=[[4, 128], [1, 4]]),
                            in_=r4[:, 4 * par:4 * par + 4])
                    for par in range(2):
                        eng = nc.sync if par == 0 else nc.scalar
                        eng.dma_start(
                            out=bc[:, par, :],
                            in_=bass.AP(tensor=d2.tensor,
                                        offset=d2.offset + par * TQ,
                                        ap=[[0, 64], [1, TQ]]))
                    for par in range(2):
                        row0 = 64 * par
                        nc.vector.tensor_mul(
                            y_tiles[(j, "sb", 3)][row0:row0 + 64, :],
                            ysts[par][0:64, :], bc[:, par, :])
                    proj3_B()

                def mk_recip(cp, par, yst, s4):
                    dmae = nc.gpsimd if par == 0 else nc.sync
                    def emit():
                        r4 = bcp.tile([128, 4], F32, tag="r4",
                                      name=f"r4_{j}_{cp}_{par}")
                        nc.vector.reciprocal(r4, s4)
                        d2 = drp.tile([1, TQ], F32, tag="d2",
                                      name=f"d2_{j}_{cp}_{par}")
                        dmae.dma_start(
                            out=bass.AP(tensor=d2.tensor, offset=d2.offset,
                                        ap=[[4, 128], [1, 4]]),
                            in_=r4)
                        bc = bcp.tile([64, TQ], F32, tag="bc",
                                      name=f"bc_{j}_{cp}_{par}")
                        dmae.dma_start(
                            out=bc,
                            in_=bass.AP(tensor=d2.tensor, offset=d2.offset,
                                        ap=[[0, 64], [1, TQ]]))
                        defer(2, mk_mul(cp, par, yst, bc))
                    return emit

                def mk_mul(cp, par, yst, bc):
                    def emit():
                        row0 = 64 * par
                        # deferred, so off the latency-critical paths; split
                        # across gpsimd/vector to keep both queues shallow
                        eng = nc.gpsimd if par == 0 else nc.vector
                        eng.tensor_mul(
                            y_tiles[(j, "sb", cp)][row0:row0 + 64, :],
                            yst[0:64, :], bc)
                        mul_count[0] += 1
                        if j == NQT - 1:
                            if mul_count[0] == 6:
                                for mo in range(8):
                                    defer(1 + mo, proj3_A(mo))
                            elif mul_count[0] == 8:
                                proj3_B()
                    return emit

                def step(k):
                    def emit():
                        gstep[0] += 1
                        flush(gstep[0])
                        if k == 0:
                            for cc in range(4):
                                y_tiles[(j, "sb", cc)] = ypool.tile(
                                    [128, TQ], BF16, tag="ysb",
                                    name=f"y_{j}_{cc}")
                        if k < len(steps):
                            emit_S(k)
                        if k >= LAG:
                            emit_PV(k - LAG)
                    return emit

                return [step(k) for k in range(len(steps) + LAG)]

            # ---------- interleaved emission ----------
            g0_early, g0_late = qkv_groups(0)
            for fn in g0_early:
                fn()
            spill = list(g0_late)
            for j in range(NQT):
                attn = attention_wave(j)
                others = list(spill)
                spill = []
                if j == 0:
                    others.append(load_wp)
                if j + 1 < NQT:
                    early, late = qkv_groups(j + 1)
                    others += early
                    spill = late
                if j == 2:
                    others += proj_groups(0)
                if j == 3:
                    others += proj_groups(1) + proj_groups(2)
                done_o = 0
                frontier = max(1, (len(attn) * 3) // 5)
                for s, fn in enumerate(attn):
                    fn()
                    want = min(len(others), (s + 1) * len(others) // frontier)
                    while done_o < want:
                        others[done_o]()
                        done_o += 1
                while done_o < len(others):
                    others[done_o]()
                    done_o += 1
            flush()  # cascade any remaining deferred norm / projection work

    nc.compile()
    _cache["nc"] = nc
    return nc


def _prep_inputs(x, W_attn, b_attn, W_proj, b_proj):
    """Host-side sharding: returns in_maps for the 8 cores."""
    import ml_dtypes
    BF = ml_dtypes.bfloat16

    x = np.ascontiguousarray(np.asarray(x, dtype=np.float32))
    W_attn = np.asarray(W_attn, dtype=np.float32)
    b_attn = np.asarray(b_attn, dtype=np.float32)
    W_proj = np.asarray(W_proj, dtype=np.float32)
    b_proj = np.asarray(b_proj, dtype=np.float32)

    bv_full = b_attn[2 * C:3 * C]
    _cache["bout_host"] = (b_proj + bv_full @ W_proj).astype(np.float32)
    tri1 = np.triu(np.ones((128, 128), dtype=np.float32))  # 1 if k<=q
    tri = np.concatenate([tri1, tri1], axis=1).astype(BF)

    xts = [np.ascontiguousarray(x[b].T.astype(BF)) for b in range(B)]
    per_g = []
    for g in range(2):
        sl = slice(512 * g, 512 * (g + 1))
        wq = W_attn[:, 0:C][:, sl] * (1.0 / np.sqrt(HD))
        wk = W_attn[:, C:2 * C][:, sl]
        wv = W_attn[:, 2 * C:3 * C][:, sl]
        bq = b_attn[0:C][sl] * (1.0 / np.sqrt(HD))
        bk = b_attn[C:2 * C][sl]
        wp = W_proj[sl, :]
        per_g.append({
            "wqk": np.ascontiguousarray(
                np.concatenate([wq, wk], axis=1).astype(BF)),
            "wv": np.ascontiguousarray(wv.astype(BF)),
            "wp": np.ascontiguousarray(wp.astype(BF)),
            "bqk": np.ascontiguousarray(np.concatenate([bq, bk])),
        })

    eye = np.ascontiguousarray(np.eye(128, dtype=np.float32))
    in_maps = []
    for b in range(B):
        for g in range(2):
            m = dict(per_g[g])
            m["xt"] = xts[b]
            m["tri"] = tri
            m["eye"] = eye
            in_maps.append(m)
    return in_maps


def run_sharded(x, W_attn, b_attn, W_proj, b_proj, trace=False):
    """Run on 8 cores; returns (output [B,T,C], BassKernelResults)."""
    from concourse.bass_utils import run_bass_kernel_spmd

    nc = _build()
    in_maps = _prep_inputs(x, W_attn, b_attn, W_proj, b_proj)
    res = run_bass_kernel_spmd(nc, in_maps, list(range(NCORES)), trace=trace)
    outs = [res.results[i]["outp"] for i in range(NCORES)]
    bout = _cache["bout_host"]
    out = np.empty((B, T, C), dtype=np.float32)
    for b in range(B):
        out[b] = (outs[2 * b] + outs[2 * b + 1]).T + bout
    return out, res


def kernel(x, W_attn, b_attn, W_proj, b_proj):
    out, _ = run_sharded(x, W_attn, b_attn, W_proj, b_proj, trace=False)
    return out
